# revision 26
# baseline (speedup 1.0000x reference)
"""AttentionalJoin kernel for 8 Trainium2 NeuronCores.

Math: the reference builds full (M x M) self-attention over M = N+1 tokens
(CLS prepended) but returns only the CLS row of the projected output.  Only
the CLS query survives, so attention collapses to a softmax-weighted token
pooling:

    q       = Wq @ cls                       (per head h: q_h)
    score_t = scale * q_h . (Wk x_t)_h  =  x_t . R[:, h],   R = scale*Wk_h^T q_h
    p       = softmax over the M tokens (scores bounded ~[-6, 6]; no max-sub)
    pooled_h = sum_t p_t x_t                 (linearity: project AFTER pooling)
    out     = proj( concat_h Wv_h pooled_h ) + proj_b

Device-side dataflow per 512-token chunk (the host ships x in BOTH layouts,
natural [tok, C] for the pooling rhs and transposed [C, tok] for the scores
rhs, both fp16 and pre-swizzled so every DMA lands as 128 contiguous
per-partition runs):

    scores   psc[128, 512] = r_rep^T @ X^T-chunk   (R replicated 16x along
             the head dim so all 128 PE output columns carry scores)
    exp      ACT: e4 = exp(psc) fp16 + per-partition Z accumulation
    E^T      4 DVE 32x32 stream-transposes with diagonal block selection --
             bank a takes token-blocks {4g+a} -- which lands E^T in exactly
             the [token, head] layout pooling needs, no PE work
    pool     ps[8, 512] += E^T-slices @ X-chunk    (4 accumulating matmuls)

The tiny tail (head-mix with Wv, proj, bias, cls-token contribution) runs
on host in fp32.  Sharding: data-parallel over batch, 2 batches per core.
"""

import ml_dtypes
import numpy as np

H = 8
C = 512
HD = C // H
B = 16
N = 2048
NCORES = 8
BPC = B // NCORES          # batches per core
TOK = BPC * N              # tokens per core (4096)
NCHUNK = TOK // 512        # 512-token compute chunks per core (8)
MAX_DRAIN_WAITS = 1        # this walrus rejects ANY inst w/ >1 sem wait
NOP_WAITS = 1

_cached = {}


def _patch_drain():
    """The container's walrus codegen rejects instructions carrying more
    than one sem wait ("Too many sync wait commands").  Split extra waits
    onto dedicated same-engine NOPs, which preserves semantics (engine
    queues are in-order)."""
    import concourse.tile as tile_mod
    from concourse import mybir
    from bass_rust import ScopedClock

    if getattr(tile_mod.TileContext, "_drain_patched", False):
        return

    orig_lower = tile_mod.TileContext._lower_ordered_insts

    def _lower_ordered_insts(self, ordered):
        nc = self.nc
        for bbname, insts in ordered.items():
            out = []
            for inst in insts:
                si = inst.sync_info
                lim = NOP_WAITS if isinstance(inst, mybir.InstNoOp) else MAX_DRAIN_WAITS
                if si is not None and si.on_wait and len(si.on_wait) > lim:
                    waits = list(si.on_wait)
                    extra, keep = waits[:-lim], waits[-lim:]
                    for i in range(0, len(extra), NOP_WAITS):
                        nop = mybir.InstNoOp(
                            name=f"waitsplit-{nc.next_id()}",
                            engine=inst.engine,
                            ins=[],
                            outs=[],
                            bass_nofuse=True,
                            sync_info=mybir.SyncInfo(
                                on_wait=extra[i : i + NOP_WAITS], on_update=[]
                            ),
                            debug=inst.debug,
                        )
                        out.append(nop)
                    inst.sync_info = mybir.SyncInfo(
                        on_wait=keep, on_update=list(si.on_update)
                    )
                out.append(inst)
            ordered[bbname] = out
        return orig_lower(self, ordered)

    tile_mod.TileContext._lower_ordered_insts = _lower_ordered_insts

    def _drain_and_barrier(self, tick_clock, wait_clock):
        nc = self.nc
        probe = mybir.InstNoOp(
            name=f"drain-wait-probe-{nc.next_id()}",
            engine=mybir.EngineType.SP,
            ins=[],
            outs=[],
        )
        wait_clock.add_sem_waits(probe, ScopedClock({None: tick_clock.global_clock}))
        waits = list(probe.sync_info.on_wait) if probe.sync_info else []
        # late-retiring output-DMA sems last, so the already-satisfied waits
        # drain through the ladder first instead of queueing behind them
        waits.sort(key=lambda w: "DMASW" in str(w))
        for w in waits:
            nop = nc.sync.nop(nofuse=True, hint="drain_wait")
            nop.ins.sync_info = mybir.SyncInfo(on_wait=[w], on_update=[])
        nc.sync.drain()

        nc.all_engine_barrier()
        popped = nc._tile_sem_poison_stack.pop()
        assert popped is self._sem_poison
        nc.clear_and_free_semaphores(list(self.sems.allocated().values()))
        nc.all_engine_barrier()

    tile_mod.TileContext._drain_and_barrier = _drain_and_barrier
    tile_mod.TileContext._drain_patched = True


def _build_module():
    import concourse.bass as bass
    import concourse.tile as tile
    from concourse import mybir

    _patch_drain()
    f16 = mybir.dt.float16
    f32 = mybir.dt.float32
    f8 = mybir.dt.float8e3
    EXP = mybir.ActivationFunctionType.Exp

    nc = bass.Bass()
    # host-pre-swizzled layouts: every DMA part is 128 contiguous
    # per-partition runs (descriptor-light HWDGE).  x^T ships as fp8-e3m4
    # (scores tolerate ~1% elementwise noise; pooling keeps fp16 x).
    x_in = nc.dram_tensor("x", [4, 128, 8, C], f16, kind="ExternalInput")
    xt_in = nc.dram_tensor("xt", [4, 128, 4, TOK // 4], f8, kind="ExternalInput")
    r_in = nc.dram_tensor("r", [128, 4, 128], f16, kind="ExternalInput")
    s_out = nc.dram_tensor("s", [BPC, H, C], f32, kind="ExternalOutput")
    z_out = nc.dram_tensor("z", [BPC, H, N // 512], f32, kind="ExternalOutput")

    with tile.TileContext(nc) as tc:
        with (
            tc.tile_pool(name="xpool", bufs=1) as xpool,
            tc.tile_pool(name="consts", bufs=1) as consts,
            tc.tile_pool(name="epool", bufs=1) as epool,
            tc.tile_pool(name="opool", bufs=2) as opool,
            tc.tile_pool(name="psc", bufs=4, space="PSUM") as psc_pool,
            tc.tile_pool(name="ps", bufs=1, space="PSUM") as ps_pool,
        ):
            r_sb = consts.tile([128, 4, 128], f16)
            nc.sync.dma_start(out=r_sb, in_=r_in[:, :, :])

            # HAM pre-warm: ~3.4us of dummy matmuls on a memset tile while
            # the first x^T parts stream in, so the real matmuls start at
            # 2.4 GHz instead of the cold 1.2 GHz
            warm_src = consts.tile([128, 512], f16)
            nc.vector.memset(warm_src, 0)
            pwarm = ps_pool.tile([8, 512], f32, tag="pwarm", name="pwarm")
            for _ in range(9):
                nc.tensor.matmul(
                    pwarm, warm_src[:, 0:8], warm_src, start=True, stop=True
                )

            # interleaved 1-MiB loads, ordered by first use:
            # xt quarter q covers scores chunks 2q, 2q+1; x part g covers
            # pooling chunks 2g, 2g+1
            # order by first use: two xt parts up front (the 4-chunk scores
            # lookahead), then x/xt interleaved
            xt_sb = [
                xpool.tile([128, 4, TOK // 4], f8, tag=f"xt{g}", name=f"xt{g}")
                for g in range(4)
            ]
            x_sb = [
                xpool.tile([128, 8, C], f16, tag=f"x{g}", name=f"x{g}")
                for g in range(4)
            ]
            for xt_g, x_g in ((0, None), (1, None), (None, 0), (2, None),
                              (None, 1), (3, None), (None, 2), (None, 3)):
                if xt_g is not None:
                    nc.sync.dma_start(out=xt_sb[xt_g], in_=xt_in[xt_g])
                if x_g is not None:
                    nc.sync.dma_start(out=x_sb[x_g], in_=x_in[x_g])

            # e4: exp(scores) with heads replicated 16x along partitions
            e4 = [
                epool.tile([128, N], f16, tag=f"e4_{b}", name=f"e4_{b}")
                for b in range(BPC)
            ]
            # zp: per-chunk partition-sums of e4 (rows 0-7 are the real Z)
            zp = [
                epool.tile([128, N // 512], f32, tag=f"zp{b}", name=f"zp{b}")
                for b in range(BPC)
            ]
            # et: E^T strips, [token-in-group, group, head(+24 replicas)]
            et = [
                epool.tile([128, 16, 32], f16, tag=f"et{b}", name=f"et{b}")
                for b in range(BPC)
            ]
            ps = [
                ps_pool.tile([H, C], f32, tag=f"ps{b}", name=f"psacc{b}")
                for b in range(BPC)
            ]

            def scores(a):
                """psc = r_rep^T @ X^T-chunk (all 128 out cols), exp+Z, then
                the DVE E^T transposes."""
                b, g = divmod(a, 4)
                hi, lo = divmod(a, 2)
                psc = psc_pool.tile([128, 512], f32, tag="psc", name=f"psc{a}")
                for q in range(4):
                    nc.tensor.matmul(
                        psc,
                        r_sb[:, q, :],
                        xt_sb[hi][:, q, lo * 512 : (lo + 1) * 512],
                        start=(q == 0),
                        stop=(q == 3),
                    )
                nc.scalar.activation(
                    out=e4[b][:, g * 512 : (g + 1) * 512],
                    in_=psc,
                    func=EXP,
                    accum_out=zp[b][:, g : g + 1],
                )
                # E^T via DVE 32x32 stream-transpose, bank a4 takes token
                # blocks {4*gl + a4} (the diagonal selection makes partition
                # 32a+i of group gl hold token 128*gl + 32a + i)
                ev = e4[b].rearrange(
                    "p (c gl four j) -> p c gl four j", c=4, four=4, j=32
                )
                for a4 in range(4):
                    nc.vector.transpose(
                        out=et[b][32 * a4 : 32 * (a4 + 1), 4 * g : 4 * (g + 1), :],
                        in_=ev[32 * a4 : 32 * (a4 + 1), g, :, a4, :],
                    )
                if g == 3:
                    # the batch's Z is complete once its last exp retires;
                    # ship it now so only s_out remains at the tail
                    nc.gpsimd.dma_start(out=z_out[b], in_=zp[b][0:H, :])

            def pool(a):
                """pooled += E^T-slices @ X (4 accumulating matmuls)."""
                b, g = divmod(a, 4)
                hi, lo = divmod(a, 2)
                for jj in range(4):
                    j = g * 4 + jj
                    nc.tensor.matmul(
                        ps[b],
                        et[b][:, j, 0:H],
                        x_sb[hi][:, lo * 4 + jj, :],
                        start=(j == 0),
                        stop=(j == 15),
                    )

            def emit_out(b):
                so = opool.tile([H, C], f32, tag=f"so{b}", name=f"so{b}")
                nc.scalar.copy(so, ps[b])
                nc.gpsimd.dma_start(out=s_out[b], in_=so)

            # software pipeline: the PE queue is in-order, so pool(a) must
            # sit BEHIND enough scores work to cover chunk a's exp + DVE
            # transpose chain (~2us).  4 chunks of scores lookahead does it.
            for a in range(4):
                scores(a)
            for a in range(4, NCHUNK):
                pool(a - 4)
                scores(a)
            pool(NCHUNK - 4)
            emit_out(0)
            pool(NCHUNK - 3)
            pool(NCHUNK - 2)
            pool(NCHUNK - 1)
            emit_out(1)

    return nc


def _get_module():
    if "nc" not in _cached:
        _cached["nc"] = _build_module()
    return _cached["nc"]


def _host_prep(cls, qkv_w):
    scale = HD ** -0.5
    c = cls.reshape(C).astype(np.float64)
    Wq = qkv_w[:C].astype(np.float64)
    Wk = qkv_w[C : 2 * C].astype(np.float64)
    q = Wq @ c
    qh = q.reshape(H, HD)
    Wkh = Wk.reshape(H, HD, C)
    R = (scale * np.einsum("hdc,hd->ch", Wkh, qh)).astype(np.float16)
    k0 = Wk @ c
    score0 = scale * np.einsum("hd,hd->h", qh, k0.reshape(H, HD))
    e0 = np.exp(score0)
    return R, e0


def _make_in_maps(inputs):
    """Per-core input dict list (shared by kernel() and the profiler)."""
    x = np.asarray(inputs["x"], dtype=np.float32)
    cls = np.asarray(inputs["cls"], dtype=np.float32)
    qkv_w = np.asarray(inputs["qkv_w"], dtype=np.float32)
    R, _ = _host_prep(cls, qkv_w)
    r_rep = np.tile(R, (1, 16)).reshape(4, 128, 128).transpose(1, 0, 2)
    r_rep = np.ascontiguousarray(r_rep)                     # [128, 4, 128]
    x16 = x.reshape(B * N, C).astype(np.float16)
    in_maps = []
    for i in range(NCORES):
        xc = x16[i * TOK : (i + 1) * TOK]                   # [4096, 512]
        # natural, swizzled: [4 parts, 128 tok-in-block, 8 blocks, 512 c]
        xn = np.ascontiguousarray(
            xc.reshape(4, 8, 128, C).transpose(0, 2, 1, 3)
        )
        # transposed, swizzled: [4 quarters of t, 128 c-in-q, 4 q, 1024 t],
        # fp8-e3m4 (scores-side only)
        xt = np.ascontiguousarray(
            xc.T.reshape(4, 128, 4, TOK // 4).transpose(2, 1, 0, 3)
        ).astype(ml_dtypes.float8_e3m4)
        in_maps.append({"x": xn, "xt": xt, "r": r_rep})
    return in_maps


def kernel(x, cls, qkv_w, proj_w, proj_b):
    from concourse.bass_utils import run_bass_kernel_spmd

    x = np.asarray(x, dtype=np.float32)
    cls = np.asarray(cls, dtype=np.float32)
    qkv_w = np.asarray(qkv_w, dtype=np.float32)
    proj_w = np.asarray(proj_w, dtype=np.float32)
    proj_b = np.asarray(proj_b, dtype=np.float32)

    _, e0 = _host_prep(cls, qkv_w)
    Wv = qkv_w[2 * C :]

    nc = _get_module()
    in_maps = _make_in_maps({"x": x, "cls": cls, "qkv_w": qkv_w})
    res = run_bass_kernel_spmd(nc, in_maps, list(range(NCORES)))
    _cached["last_results"] = res

    s_dev = np.concatenate([res.results[i]["s"] for i in range(NCORES)], axis=0)
    z_dev = np.concatenate(
        [res.results[i]["z"].sum(axis=-1) for i in range(NCORES)], axis=0
    )

    # add the CLS token's own contribution, normalize, head-mix + proj
    cf = cls.reshape(C)
    s_full = s_dev + (e0[:, None] * cf[None, :]).astype(np.float32)[None]
    z_full = z_dev + e0.astype(np.float32)[None]
    v = s_full / z_full[:, :, None]
    o = np.einsum("hdc,bhc->bhd", Wv.reshape(H, HD, C), v).reshape(B, C)
    y = o @ proj_w.T + proj_b
    return y.astype(np.float32)
